# revision 2
# baseline (speedup 1.0000x reference)
"""Gated axial attention (height) Trainium2 kernel.

N,C,H,W = 16,128,128,128. 8 NeuronCores, data-parallel over batch N
(2 batches per core). All math per (core, batch n):

  q~ = (Wq/d) @ x          [c,(i,j)]   (d = sqrt(C))
  k  =  Wk    @ x          [c,(h,j)]
  vT_j[h,c] = sum_c' Gv1*Wv[c,c'] x[c',h,j]      (per-j matmul, transposed v)
  Eq = exp(q~_j^T k_j)     stored [h,(i,j)] via strided-dest ACT
  Sr_i = (Gq*rq_i)^T q~_i + (Gk/d*rk_i)^T k_i    (per-i matmul, PSUM accum)
  E  = Eq * exp(Sr)        (DVE mul, in-place into Eq)
  sig[h,i] = sum_j E ; R = 1/sig ; Wn = E * R[h,i]
  out_j[c,i] += vT_j^T Wn_j   (per-j matmul -> strided add)
  out_i[c,j] += rv_i^T Wn_i   (per-i matmul -> contiguous copy)
"""

import numpy as np
import ml_dtypes

import concourse.bass as bass
import concourse.tile as tile
from concourse import bacc, mybir
from concourse.bass_utils import run_bass_kernel_spmd

N, C, H, W = 16, 128, 128, 128
HW = H * W
N_CORES = 8
NPC = N // N_CORES  # batches per core
BF16 = mybir.dt.bfloat16
F32 = mybir.dt.float32
ICHUNK = 32  # i-block streamed for rq/rk/rv

_PROG = None


def _build():
    nc = bacc.Bacc("TRN2", target_bir_lowering=False, debug=False,
                   num_devices=N_CORES)
    x_ap = nc.dram_tensor("x2", [NPC, C, HW], BF16, kind="ExternalInput").ap()
    wq_ap = nc.dram_tensor("wqt", [C, C], BF16, kind="ExternalInput").ap()
    wk_ap = nc.dram_tensor("wkt", [C, C], BF16, kind="ExternalInput").ap()
    wv_ap = nc.dram_tensor("wvt", [C, C], BF16, kind="ExternalInput").ap()
    rq_ap = nc.dram_tensor("rqh", [C, HW], BF16, kind="ExternalInput").ap()
    rk_ap = nc.dram_tensor("rkh", [C, HW], BF16, kind="ExternalInput").ap()
    rv_ap = nc.dram_tensor("rvh", [H, H * C], BF16, kind="ExternalInput").ap()
    y_ap = nc.dram_tensor("y", [NPC, C, HW], BF16, kind="ExternalOutput").ap()

    from contextlib import ExitStack
    with tile.TileContext(nc) as tc, ExitStack() as ctx:
        wpool = ctx.enter_context(tc.tile_pool(name="w", bufs=1))
        big = ctx.enter_context(tc.tile_pool(name="big", bufs=1))
        chunk = ctx.enter_context(tc.tile_pool(name="chunk", bufs=3))
        small = ctx.enter_context(tc.tile_pool(name="small", bufs=2))
        pp = ctx.enter_context(tc.tile_pool(name="pp", bufs=6, space="PSUM"))

        wq = wpool.tile([C, C], BF16, tag="wq")
        wk = wpool.tile([C, C], BF16, tag="wk")
        wv = wpool.tile([C, C], BF16, tag="wv")
        nc.sync.dma_start(wq[:], wq_ap[:])
        nc.sync.dma_start(wk[:], wk_ap[:])
        nc.sync.dma_start(wv[:], wv_ap[:])

        for n in range(NPC):
            # ---- stage A: load x, project q/k, build vT --------------------
            xb = big.tile([C, HW], BF16, tag="x_eq")     # also Eq's slot later
            for s in range(4):
                nc.sync.dma_start(xb[:, s * 4096:(s + 1) * 4096],
                                  x_ap[n][:, s * 4096:(s + 1) * 4096])
            qb = big.tile([C, HW], BF16, tag="qb")
            kb = big.tile([C, HW], BF16, tag="kb")
            for s in range(HW // 512):
                ps = pp.tile([128, 512], F32, tag="ps")
                nc.tensor.matmul(ps[:], wq[:], xb[:, s * 512:(s + 1) * 512])
                nc.scalar.copy(qb[:, s * 512:(s + 1) * 512], ps[:])
                ps2 = pp.tile([128, 512], F32, tag="ps")
                nc.tensor.matmul(ps2[:], wk[:], xb[:, s * 512:(s + 1) * 512])
                nc.scalar.copy(kb[:, s * 512:(s + 1) * 512], ps2[:])
            vT = big.tile([H, W * C], BF16, tag="vT")    # [h,(j,c)]
            for j0 in range(0, W, 4):
                ps = pp.tile([128, 512], F32, tag="ps")
                for jj in range(4):
                    j = j0 + jj
                    nc.tensor.matmul(ps[:, jj * C:(jj + 1) * C],
                                     xb[:, j::W], wv[:])
                nc.vector.tensor_copy(vT[:, j0 * C:(j0 + 4) * C], ps[:])

            # ---- stage C: qk -> Eq = exp(qk), layout [h,(i,j)] -------------
            Eq = big.tile([H, HW], BF16, tag="x_eq")
            Eq_ji = Eq[:].rearrange("p (i j) -> p j i", j=W)
            for j0 in range(0, W, 4):
                ps = pp.tile([128, 512], F32, tag="ps")
                for jj in range(4):
                    j = j0 + jj
                    nc.tensor.matmul(ps[:, jj * H:(jj + 1) * H],
                                     kb[:, j::W], qb[:, j::W])
                nc.scalar.activation(Eq_ji[:, j0:j0 + 4, :], ps[:],
                                     mybir.ActivationFunctionType.Exp)

            # ---- stage B: Sr -> E = Eq * exp(Sr) ---------------------------
            for i0 in range(0, H, ICHUNK):
                rqc = chunk.tile([C, ICHUNK * H], BF16, tag="chunk")
                nc.sync.dma_start(rqc[:], rq_ap[:, i0 * H:(i0 + ICHUNK) * H])
                rkc = chunk.tile([C, ICHUNK * H], BF16, tag="chunk")
                nc.sync.dma_start(rkc[:], rk_ap[:, i0 * H:(i0 + ICHUNK) * H])
                for i1 in range(0, ICHUNK, 4):
                    ps = pp.tile([128, 512], F32, tag="ps")
                    for ii in range(4):
                        i = i0 + i1 + ii
                        il = i1 + ii
                        nc.tensor.matmul(ps[:, ii * W:(ii + 1) * W],
                                         rqc[:, il * H:(il + 1) * H],
                                         qb[:, i * W:(i + 1) * W],
                                         start=True, stop=False)
                        nc.tensor.matmul(ps[:, ii * W:(ii + 1) * W],
                                         rkc[:, il * H:(il + 1) * H],
                                         kb[:, i * W:(i + 1) * W],
                                         start=False, stop=True)
                    st = small.tile([128, 512], BF16, tag="stemp")
                    nc.scalar.activation(st[:], ps[:],
                                         mybir.ActivationFunctionType.Exp)
                    i = i0 + i1
                    nc.vector.tensor_mul(Eq[:, i * W:(i + 4) * W],
                                         Eq[:, i * W:(i + 4) * W], st[:])

            # ---- softmax denominator over j --------------------------------
            sig = small.tile([H, H], F32, tag="sig")
            nc.vector.tensor_reduce(
                sig[:], Eq[:].rearrange("p (i j) -> p i j", j=W),
                axis=mybir.AxisListType.X, op=mybir.AluOpType.add)
            rec = small.tile([H, H], F32, tag="rec")
            nc.vector.reciprocal(rec[:], sig[:])
            for i in range(H):
                nc.vector.tensor_scalar_mul(Eq[:, i * W:(i + 1) * W],
                                            Eq[:, i * W:(i + 1) * W],
                                            rec[:, i:i + 1])

            # ---- stage F: outputs ------------------------------------------
            outb = big.tile([C, HW], BF16, tag="out")
            Wn_ij = Eq[:].rearrange("p (i j) -> p i j", j=W)
            # out2 (per-i, contiguous) interleaved with out1 (per-j, strided)
            for i0 in range(0, H, ICHUNK):
                rvc = chunk.tile([H, ICHUNK * C], BF16, tag="chunk")
                nc.sync.dma_start(rvc[:], rv_ap[:, i0 * C:(i0 + ICHUNK) * C])
                for i1 in range(0, ICHUNK, 4):
                    ps = pp.tile([128, 512], F32, tag="ps")
                    for ii in range(4):
                        i = i0 + i1 + ii
                        il = i1 + ii
                        nc.tensor.matmul(ps[:, ii * W:(ii + 1) * W],
                                         rvc[:, il * C:(il + 1) * C],
                                         Eq[:, i * W:(i + 1) * W])
                    i = i0 + i1
                    nc.scalar.copy(outb[:, i * W:(i + 4) * W], ps[:])
            out_ji = outb[:].rearrange("p (i j) -> p j i", j=W)
            for j0 in range(0, W, 4):
                ps = pp.tile([128, 512], F32, tag="ps")
                for jj in range(4):
                    j = j0 + jj
                    nc.tensor.matmul(ps[:, jj * H:(jj + 1) * H],
                                     vT[:, j * C:(j + 1) * C],
                                     Wn_ij[:, :, j])
                nc.vector.tensor_add(
                    out_ji[:, j0:j0 + 4, :], out_ji[:, j0:j0 + 4, :],
                    ps[:].rearrange("p (a b) -> p a b", b=H))
            for s in range(4):
                nc.sync.dma_start(y_ap[n][:, s * 4096:(s + 1) * 4096],
                                  outb[:, s * 4096:(s + 1) * 4096])

    nc.compile()
    return nc


def _get_prog():
    global _PROG
    if _PROG is None:
        _PROG = _build()
    return _PROG


def _prep_inputs(x, Wq, Wk, Wv, rq, rk, rv, Gq, Gk, Gv1, Gv2):
    bf = ml_dtypes.bfloat16
    d = np.float32(np.sqrt(C))
    wqt = np.ascontiguousarray((Wq / d).T).astype(bf)
    wkt = np.ascontiguousarray(Wk.T).astype(bf)
    wvt = np.ascontiguousarray((Gv1[0] * Wv).T).astype(bf)
    rqh = np.ascontiguousarray((Gq[0] * rq).transpose(0, 2, 1)).reshape(C, HW).astype(bf)
    rkh = np.ascontiguousarray((Gk[0] / d * rk).transpose(0, 2, 1)).reshape(C, HW).astype(bf)
    rvh = np.ascontiguousarray((Gv2[0] * rv).transpose(1, 2, 0)).reshape(H, H * C).astype(bf)
    xb = np.ascontiguousarray(x).reshape(N, C, HW).astype(bf)
    return xb, wqt, wkt, wvt, rqh, rkh, rvh


def kernel(x, Wq, Wk, Wv, rq, rk, rv, Gq, Gk, Gv1, Gv2):
    x = np.asarray(x, np.float32)
    xb, wqt, wkt, wvt, rqh, rkh, rvh = _prep_inputs(
        np.asarray(x, np.float32), np.asarray(Wq, np.float32),
        np.asarray(Wk, np.float32), np.asarray(Wv, np.float32),
        np.asarray(rq, np.float32), np.asarray(rk, np.float32),
        np.asarray(rv, np.float32), np.asarray(Gq, np.float32),
        np.asarray(Gk, np.float32), np.asarray(Gv1, np.float32),
        np.asarray(Gv2, np.float32))
    nc = _get_prog()
    in_maps = []
    for c in range(N_CORES):
        in_maps.append({
            "x2": xb[c * NPC:(c + 1) * NPC], "wqt": wqt, "wkt": wkt,
            "wvt": wvt, "rqh": rqh, "rkh": rkh, "rvh": rvh,
        })
    res = run_bass_kernel_spmd(nc, in_maps, list(range(N_CORES)))
    out = np.empty((N, C, HW), np.float32)
    for c in range(N_CORES):
        out[c * NPC:(c + 1) * NPC] = res.results[c]["y"].astype(np.float32)
    return out.reshape(N, C, H, W)
